# revision 6
# baseline (speedup 1.0000x reference)
import functools

import numpy as np

import concourse.tile_sem_assignment as _tsa

# This walrus encodes at most one semaphore wait per instruction, so the
# TileContext exit drain cannot wait on Tile's default 8 round-robin HWDGE
# completion lanes. Pin all HW-DGE DMAs to a single lane; this kernel has
# no intermediate consumers, so the shared lane adds no false serialization.
_tsa.NUM_HWDGE_SEMS = 1

import concourse.bass as bass
import concourse.mybir as mybir
from concourse.bass_utils import run_bass_kernel_spmd
from concourse.tile import TileContext

B, T, F = 256, 512, 256
NCORES = 8
BS = B // NCORES

LAST_RESULT = None
LAST_RUN = None


def _keep_runs(mask: np.ndarray):
    keep = ~mask.astype(bool)
    runs = []
    t = 0
    while t < T:
        if keep[t]:
            t0 = t
            while t < T and keep[t]:
                t += 1
            runs.append((t0, t))
        else:
            t += 1
    return tuple(runs)


@functools.lru_cache(maxsize=4)
def _build_nc(runs):
    nc = bass.Bass(target_bir_lowering=False)
    ins = {}
    outs = {}
    for name in ("xd", "xt", "xs"):
        ins[name] = nc.dram_tensor(name, [BS, T, F], mybir.dt.float32, kind="ExternalInput")
    for name in ("zd", "zt", "zs"):
        outs[name] = nc.dram_tensor(name, [BS, T, F], mybir.dt.float32, kind="ExternalOutput")

    with TileContext(nc):
        engines = (nc.sync, nc.scalar)
        i = 0
        for xn, zn in (("xd", "zd"), ("xt", "zt"), ("xs", "zs")):
            x, z = ins[xn], outs[zn]
            for t0, t1 in runs:
                engines[i % 2].dma_start(out=z[:, t0:t1, :], in_=x[:, t0:t1, :])
                i += 1
    return nc


def kernel(x_dist, x_tre, x_sea, mask):
    global LAST_RESULT, LAST_RUN
    runs = _keep_runs(np.asarray(mask))
    xs = {
        "xd": np.asarray(x_dist, dtype=np.float32),
        "xt": np.asarray(x_tre, dtype=np.float32),
        "xs": np.asarray(x_sea, dtype=np.float32),
    }
    if not runs:
        zero = np.zeros((B, T, F), np.float32)
        return zero, zero.copy(), zero.copy()

    in_maps = [
        {k: np.ascontiguousarray(v[c * BS:(c + 1) * BS]) for k, v in xs.items()}
        for c in range(NCORES)
    ]

    nc = _build_nc(runs)
    LAST_RUN = (nc, in_maps)
    res = run_bass_kernel_spmd(nc, in_maps, core_ids=list(range(NCORES)))
    LAST_RESULT = res

    out = []
    for name in ("zd", "zt", "zs"):
        out.append(
            np.concatenate([res.results[c][name] for c in range(NCORES)], axis=0)
        )
    return tuple(out)


# revision 7
# speedup vs baseline: 2.9514x; 2.9514x over previous
import functools

import numpy as np

import concourse.bass as bass
import concourse.mybir as mybir
from concourse.bass_utils import run_bass_kernel_spmd
from concourse.tile import TileContext
from concourse.vector_clock import ScopedClock

B, T, F = 256, 512, 256
NCORES = 8
BS = B // NCORES

LAST_RESULT = None
LAST_RUN = None


def _split_drain_and_barrier(self, tick_clock, wait_clock):
    # This walrus encodes at most one semaphore wait per instruction, so the
    # stock exit drain (one wait per HWDGE completion lane) fails codegen.
    # Emit one single-wait drain per lane instead.
    drain_inst = self.nc.sync.drain()
    wait_clock.add_sem_waits(
        drain_inst.ins, ScopedClock({None: tick_clock.global_clock})
    )
    si = drain_inst.ins.sync_info
    waits = list(si.on_wait or [])
    if len(waits) > 1:
        si.on_wait = waits[:1]
        for w in waits[1:]:
            d2 = self.nc.sync.drain()
            si2 = d2.ins.sync_info
            if si2 is None:
                d2.ins.sync_info = mybir.SyncInfo(on_wait=[w], on_update=[])
            else:
                si2.on_wait = [w]

    self.nc.all_engine_barrier()
    assert self.sems is not None
    popped = self.nc._tile_sem_poison_stack.pop()
    assert popped is self._sem_poison
    self.nc.clear_and_free_semaphores(list(self.sems.allocated().values()))
    self.nc.all_engine_barrier()


TileContext._drain_and_barrier = _split_drain_and_barrier


def _keep_runs(mask: np.ndarray):
    keep = ~mask.astype(bool)
    runs = []
    t = 0
    while t < T:
        if keep[t]:
            t0 = t
            while t < T and keep[t]:
                t += 1
            runs.append((t0, t))
        else:
            t += 1
    return tuple(runs)


@functools.lru_cache(maxsize=4)
def _build_nc(runs):
    nc = bass.Bass(target_bir_lowering=False)
    x = nc.dram_tensor("x", [3, BS, T, F], mybir.dt.float32, kind="ExternalInput")
    z = nc.dram_tensor("z", [3, BS, T, F], mybir.dt.float32, kind="ExternalOutput")

    with TileContext(nc):
        engines = (nc.sync, nc.scalar)
        for i, (t0, t1) in enumerate(runs):
            engines[i % 2].dma_start(out=z[:, :, t0:t1, :], in_=x[:, :, t0:t1, :])
    return nc


def kernel(x_dist, x_tre, x_sea, mask):
    global LAST_RESULT, LAST_RUN
    runs = _keep_runs(np.asarray(mask))
    full = np.stack(
        [
            np.asarray(x_dist, dtype=np.float32),
            np.asarray(x_tre, dtype=np.float32),
            np.asarray(x_sea, dtype=np.float32),
        ]
    )
    if not runs:
        zero = np.zeros((B, T, F), np.float32)
        return zero, zero.copy(), zero.copy()

    in_maps = [
        {"x": np.ascontiguousarray(full[:, c * BS:(c + 1) * BS])}
        for c in range(NCORES)
    ]

    nc = _build_nc(runs)
    LAST_RUN = (nc, in_maps)
    res = run_bass_kernel_spmd(nc, in_maps, core_ids=list(range(NCORES)))
    LAST_RESULT = res

    z = np.concatenate([res.results[c]["z"] for c in range(NCORES)], axis=1)
    return z[0], z[1], z[2]
